# revision 33
# baseline (speedup 1.0000x reference)
"""Trainium2 Bass kernel for nn_EncoderGRU (B=128, T=512, D=64, H=512).

Strategy
--------
Pure data-parallel over batch: 8 cores x 16 batch rows each. The T=512 GRU
recurrence is inherently sequential, so per-step serial latency is everything.

Per step the PE computes gate pre-activations with h as the *stationary*
operand and weights as the *moving* operand, 4-way column tiling: strip q
(PSUM partitions 32q..) accumulates gate block [r_q | zbar_q | hn_q | in_q].
The z gate is negated host-side so sigmoid directly yields zbar = 1-z.

v2 restructure (vs the 4.18ms baseline):
 - PSUM accumulation split into two groups per strip: cols 0:256 (r,zbar) and
   cols 256:512 (hn,in). The r/zbar matmuls stop first so the sigmoid runs
   concurrently with the hn/in matmuls.
 - Dual-z: a second sigmoid with scale=-1 yields z, so
   h' = zbar*n + z*h with p2 = z*h computed during the tanh. The two products
   are transposed by two small PE matmuls accumulating into one PSUM tile,
   giving h'^T directly; the batch-layout h' = p1+p2 runs off the critical
   path. This removes the big LDWEIGHTS(h_bl)+transpose and two vector ops
   from the serial chain.
 - Filler matmuls keep the PE continuously busy so it holds its max p-state
   (otherwise matmuls run at ~1.2GHz instead of 2.4GHz).
 - Zero per-step DMAs: x is preloaded to SBUF in (t,b)-major once, the
   delta/bias stationary rows are built host-side, and the output is written
   in kernel-native [D, T*BL] layout with one DMA per 32-step block (the
   host reorders). The baseline serialized ~500 DMAs on the Sync engine.

samp_mask is read host-side and the kernel is specialized per step
(teacher forcing vs autoregressive) at build time.
"""

import sys
import numpy as np

sys.path.insert(0, "/opt/trn_rl_repo")

B, T, D, H = 128, 512, 64, 512
NC = 8            # cores
BL = B // NC      # local batch = 16
OUT_BLK = 32      # steps per output-projection block
RING = 64         # h-history ring slots (must be multiple of OUT_BLK)

_CACHE = {}


def _ap(base, offset_add, ap_dims):
    """Clone an AP keeping the partition dim, replacing the free dims (SBUF)."""
    import concourse.bass as bass
    return bass.AP(
        tensor=base.tensor,
        offset=base.offset + offset_add,
        ap=[base.ap[0]] + ap_dims,
    )


def _apd(base, offset_add, ap_dims):
    """Clone a DRAM AP with fully custom dims."""
    import concourse.bass as bass
    return bass.AP(
        tensor=base.tensor,
        offset=base.offset + offset_add,
        ap=ap_dims,
    )


def _patch_drain_wait_limit():
    """The walrus build in this image rejects >1 sync wait on a CTRL/Drain
    instruction ("Too many sync wait commands"). Spread the kernel-tail
    drain's waits across multiple drain instructions."""
    import bass_rust
    import concourse.tile as tile
    from concourse.vector_clock import ScopedClock

    if getattr(tile.TileContext, "_drain_patched", False):
        return
    MAXW = 1

    def _drain_and_barrier(self, tick_clock, wait_clock):
        drain_inst = self.nc.sync.drain()
        wait_clock.add_sem_waits(
            drain_inst.ins, ScopedClock({None: tick_clock.global_clock})
        )
        si = drain_inst.ins.sync_info
        waits = list(si.on_wait or [])
        if len(waits) > MAXW:
            si.on_wait = waits[:MAXW]
            rest = waits[MAXW:]
            for i in range(0, len(rest), MAXW):
                d2 = self.nc.sync.drain()
                chunk = rest[i:i + MAXW]
                si2 = d2.ins.sync_info
                if si2 is None:
                    d2.ins.sync_info = bass_rust.SyncInfo(
                        on_wait=chunk, on_update=[])
                else:
                    si2.on_wait = chunk
        self.nc.all_engine_barrier()
        assert self.sems is not None
        popped = self.nc._tile_sem_poison_stack.pop()
        assert popped is self._sem_poison
        self.nc.clear_and_free_semaphores(list(self.sems.allocated().values()))
        self.nc.all_engine_barrier()

    tile.TileContext._drain_and_barrier = _drain_and_barrier
    tile.TileContext._drain_patched = True


def _split_multi_waits(nc):
    """The walrus build in this image accepts at most ONE sync wait per
    instruction. Hoist extra waits onto preceding single-wait NoOps on the
    same engine (engine sequencers execute in order, so semantics hold)."""
    import bass_rust
    from concourse import mybir

    n_new = 0
    for fn in nc.m.functions:
        for bb in fn.blocks:
            out = []
            for inst in bb.instructions:
                si = inst.sync_info
                waits = list(si.on_wait) if si and si.on_wait else []
                if len(waits) > 1:
                    for i, w in enumerate(waits[:-1]):
                        nop = mybir.InstNoOp(
                            name=f"{inst.name}-w{i}",
                            engine=inst.engine,
                            ins=[],
                            outs=[],
                            sync_info=bass_rust.SyncInfo(
                                on_wait=[w], on_update=[]),
                        )
                        out.append(nop)
                        n_new += 1
                    si.on_wait = waits[-1:]
                out.append(inst)
            bb.instructions = out
    return n_new


def _build(samp_mask: np.ndarray, t_run: int = T, split_waits: bool = True):
    """Build the Bass module (specialized on samp_mask). Returns nc."""
    import concourse.bass as bass
    import concourse.tile as tile
    from concourse import mybir

    _patch_drain_wait_limit()

    f32 = mybir.dt.float32
    bf16 = mybir.dt.bfloat16
    AF = mybir.ActivationFunctionType

    mask = [bool(v) for v in samp_mask]
    n_blocks = t_run // OUT_BLK

    nc = bass.Bass()

    # ---- DRAM parameters (host supplies preprocessed layouts) ----
    xall_d = nc.declare_dram_parameter("x_all", [D, T * BL], bf16, isOutput=False)
    dstat_d = nc.declare_dram_parameter("dstat", [2, T * 32], bf16, isOutput=False)
    war_d = nc.declare_dram_parameter("w_ar", [128, 4, 2048], bf16, isOutput=False)
    wtf_d = nc.declare_dram_parameter("w_tf", [128, 4, 1536], bf16, isOutput=False)
    w4ar_d = nc.declare_dram_parameter("w4_ar", [2, 2048], bf16, isOutput=False)
    w4tf_d = nc.declare_dram_parameter("w4_tf", [2, 2048], bf16, isOutput=False)
    wx_d = nc.declare_dram_parameter("w_x", [64, 4, 384], bf16, isOutput=False)
    wout_d = nc.declare_dram_parameter("w_out_k", [128, 4, D], bf16, isOutput=False)
    bout_d = nc.declare_dram_parameter("b_out_c", [D, 1], f32, isOutput=False)
    ident_d = nc.declare_dram_parameter("ident", [128, 128], f32, isOutput=False)
    out_d = nc.declare_dram_parameter("out", [D, T * BL], f32, isOutput=True)

    with tile.TileContext(nc) as tc:
        with (
            tc.tile_pool(name="const", bufs=1) as consts,
            tc.tile_pool(name="apsA", bufs=2, space="PSUM") as apsA_pool,
            tc.tile_pool(name="apsBC", bufs=2, space="PSUM") as apsBC_pool,
            tc.tile_pool(name="tpsum", bufs=1, space="PSUM") as tpsum_pool,
            tc.tile_pool(name="opsum", bufs=1, space="PSUM") as opsum_pool,
            tc.tile_pool(name="chain", bufs=3) as chain,
            tc.tile_pool(name="pp", bufs=3) as pp_pool,
            tc.tile_pool(name="hbl", bufs=3) as hbl_pool,
            tc.tile_pool(name="osb", bufs=2) as osb_pool,
        ):
            # ---- constants into SBUF ----
            w_ar = consts.tile([128, 4, 2048], bf16)
            nc.sync.dma_start(out=w_ar[:], in_=war_d[:])
            w_tf = consts.tile([128, 4, 1536], bf16)
            nc.sync.dma_start(out=w_tf[:], in_=wtf_d[:])
            w4_ar = consts.tile([2, 2048], bf16)
            nc.sync.dma_start(out=w4_ar[:], in_=w4ar_d[:])
            w4_tf = consts.tile([2, 2048], bf16)
            nc.sync.dma_start(out=w4_tf[:], in_=w4tf_d[:])
            w_x = consts.tile([64, 4, 384], bf16)
            nc.sync.dma_start(out=w_x[:], in_=wx_d[:])
            w_out_k = consts.tile([128, 4, D], bf16)
            nc.sync.dma_start(out=w_out_k[:], in_=wout_d[:])
            b_out_c = consts.tile([D, 1], f32)
            nc.sync.dma_start(out=b_out_c[:], in_=bout_d[:])
            ident = consts.tile([128, 128], f32)
            nc.sync.dma_start(out=ident[:], in_=ident_d[:])
            # x in (t,b)-major: xall[d, t*BL+b] = x[b,t,d]
            xall = consts.tile([D, T * BL], bf16)
            nc.sync.dma_start(out=xall[:], in_=xall_d[:])
            # dstat row0: per-step last-channel values (delta_t for TF steps,
            # x[b,t,63] for AR steps), t-major (t,b); row1 = ones (bias row)
            dstat = consts.tile([2, T * 32], bf16)
            nc.sync.dma_start(out=dstat[:], in_=dstat_d[:])

            # h-history ring: slot (t % RING) holds h_state(t+1)^T = h_arr[t]^T
            # slot layout: 4 k-groups x 32 cols; cols 16-31 of each group are
            # zero so 32-col stationaries write full 32-row PSUM strips
            h_ring = consts.tile([128, RING * 128], bf16)
            nc.vector.memset(h_ring[:], 0.0)
            zeros_h = consts.tile([128, 128], bf16)
            nc.vector.memset(zeros_h[:], 0.0)
            h0_bl = consts.tile([112, 128], f32)
            nc.vector.memset(h0_bl[:], 0.0)

            STRIPS = (0, 32, 64, 96)
            hv = h_ring[:].rearrange("p (t g) -> p t g", g=128)

            def emit_out_block(blk):
                """project h_arr steps [blk*OUT_BLK, (blk+1)*OUT_BLK) -> out"""
                t0 = blk * OUT_BLK
                r0 = t0 % RING
                opsum = opsum_pool.tile([D, OUT_BLK * BL], f32)
                for kb in range(4):
                    rhs = hv[:, r0:r0 + OUT_BLK, 32 * kb:32 * kb + 16]
                    nc.tensor.matmul(
                        opsum[:],
                        w_out_k[:, kb, :],
                        rhs,
                        start=(kb == 0),
                        stop=(kb == 3),
                    )
                o_sb = osb_pool.tile([D, OUT_BLK * BL], f32)
                nc.vector.tensor_scalar_add(o_sb[:], opsum[:], b_out_c[:])
                # one contiguous DMA per block; host reorders afterwards
                dst = _apd(out_d[:], t0 * BL,
                           [[T * BL, D], [1, OUT_BLK * BL]])
                nc.sync.dma_start(out=dst, in_=o_sb[:])

            def emit_transpose(h_bl_t, tpsum_t):
                """tpsum = h_bl^T (PE)"""
                nc.tensor.matmul(
                    tpsum_t[:], h_bl_t[:], ident[0:112, 0:112],
                    start=True, stop=True, is_transpose=True,
                    skip_group_check=True,
                )

            def emit_cast(tpsum_t, slot):
                hsrc = _ap(tpsum_t[:], 0, [[32, 4], [1, 16]])
                hdst = _ap(h_ring[0:128, slot * 128:(slot + 1) * 128], 0,
                           [[32, 4], [1, 16]])
                nc.vector.tensor_copy(out=hdst, in_=hsrc)

            def emit_head(t):
                """h-independent matmuls for step t: K4 (delta+bias) starts
                both psum tiles; TF steps add the x-side matmuls."""
                tf = mask[t]
                w4 = w4_tf if tf else w4_ar
                pA = apsA_pool.tile([128, 256], f32)
                pBC = apsBC_pool.tile([128, 256], f32)
                for dst, c0 in ((pA, 0), (pBC, 256)):
                    for q, sp in enumerate(STRIPS):
                        nc.tensor.matmul(
                            dst[sp:sp + 32, 0:256],
                            dstat[:, t * 32:(t + 1) * 32],
                            w4[:, 512 * q + c0:512 * q + c0 + 256],
                            start=True, stop=False,
                            tile_position=(0, sp),
                            skip_group_check=True,
                        )
                if tf:
                    xin = xall[:, t * BL:(t + 1) * BL]
                    for q, sp in enumerate(STRIPS):
                        nc.tensor.matmul(
                            pA[sp:sp + BL, 0:256],
                            xin,
                            w_x[:, q, 0:256],
                            start=False, stop=False,
                            tile_position=(0, sp),
                            skip_group_check=True,
                        )
                        nc.tensor.matmul(
                            pBC[sp:sp + BL, 128:256],
                            xin,
                            w_x[:, q, 256:384],
                            start=False, stop=False,
                            tile_position=(0, sp),
                            skip_group_check=True,
                        )
                return pA, pBC

            h_bl_prev = h0_bl
            cur = emit_head(0)

            for t in range(t_run):
                tf = mask[t]
                pA, pBC = cur

                # ---------- transpose h'(t-1) into psum, cast to ring ------
                if t > 0:
                    tpsum_t = tpsum_pool.tile([128, 112], f32)
                    emit_transpose(h_bl_prev, tpsum_t)
                    emit_cast(tpsum_t, (t - 1) % RING)
                    h_stat = h_ring[:, ((t - 1) % RING) * 128:
                                    ((t - 1) % RING + 1) * 128]
                else:
                    h_stat = zeros_h[:]

                # ---------- recurrent matmuls: rz group, then hn/in -------
                wmov, W = (w_tf, 384) if tf else (w_ar, 512)
                for k in range(4):
                    lhsT = h_stat[:, 32 * k:32 * k + 32]
                    for q, sp in enumerate(STRIPS):
                        nc.tensor.matmul(
                            pA[sp:sp + 32, 0:256],
                            lhsT,
                            wmov[:, k, W * q:W * q + 256],
                            start=False, stop=(k == 3),
                            tile_position=(0, sp),
                            skip_group_check=True,
                        )
                # AR streams hn|in as one 256-wide group (4-way strip
                # concurrency); TF has only the 128-wide hn columns
                wid = 128 if tf else 256
                for k in range(4):
                    lhsT = h_stat[:, 32 * k:32 * k + 32]
                    for q, sp in enumerate(STRIPS):
                        nc.tensor.matmul(
                            pBC[sp:sp + 32, 0:wid],
                            lhsT,
                            wmov[:, k, W * q + 256:W * q + 256 + wid],
                            start=False, stop=(k == 3),
                            tile_position=(0, sp),
                            skip_group_check=True,
                        )

                # next step's h-independent matmuls run during this chain
                if t + 1 < t_run:
                    cur = emit_head(t + 1)

                # ---------- output projection (PE busy during chain) -------
                if t % OUT_BLK == 0 and t >= OUT_BLK:
                    emit_out_block(t // OUT_BLK - 1)

                # ---------- gate chain ----------
                sig_r = chain.tile([112, 128], f32, tag="sig_r")
                nc.scalar.activation(sig_r[:], pA[0:112, 0:128], AF.Sigmoid)
                sig_z = chain.tile([112, 128], f32, tag="sig_z")
                nc.scalar.activation(sig_z[:], pA[0:112, 128:256], AF.Sigmoid)
                pre1 = chain.tile([112, 128], f32, tag="pre1")
                nc.vector.tensor_mul(pre1[:], sig_r[:], pBC[0:112, 0:128])
                pre2 = chain.tile([112, 128], f32, tag="pre2")
                nc.vector.tensor_add(pre2[:], pre1[:], pBC[0:112, 128:256])
                n_sb = chain.tile([112, 128], f32, tag="n")
                nc.scalar.activation(n_sb[:], pre2[:], AF.Tanh)
                # p2m = (zbar-1)*h = -z*h; exact z, fused, off critical path
                p2_t = pp_pool.tile([112, 128], f32, tag="p2")
                nc.vector.scalar_tensor_tensor(
                    p2_t[:], sig_z[:], 1.0, h_bl_prev[:],
                    op0=mybir.AluOpType.subtract, op1=mybir.AluOpType.mult)
                p1_t = pp_pool.tile([112, 128], f32, tag="p1")
                nc.vector.tensor_mul(p1_t[:], sig_z[:], n_sb[:])
                h_bl = hbl_pool.tile([112, 128], f32, tag="hbl")
                nc.vector.tensor_sub(h_bl[:], p1_t[:], p2_t[:])
                h_bl_prev = h_bl

            # final transpose + cast + last output block
            tpsum_t = tpsum_pool.tile([128, 112], f32)
            emit_transpose(h_bl_prev, tpsum_t)
            emit_cast(tpsum_t, (t_run - 1) % RING)
            if n_blocks > 0:
                emit_out_block(n_blocks - 1)

    if split_waits:
        _split_multi_waits(nc)
    return nc


def _preprocess(W_ih, W_hh, b_ih, b_hh, W_out, b_out):
    """Host-side weight folding into the layouts the kernel expects."""
    f = np.float32
    W_ih = np.asarray(W_ih, f); W_hh = np.asarray(W_hh, f)
    b_ih = np.asarray(b_ih, f); b_hh = np.asarray(b_hh, f)
    W_out = np.asarray(W_out, f); b_out = np.asarray(b_out, f)

    Wih_d = W_ih[:, :D]          # (3H, D)
    w_last = W_ih[:, D]          # (3H,)
    W_gi = W_out @ Wih_d.T       # (H, 3H)
    c_ar = b_out @ Wih_d.T + b_ih

    sl = {"r": slice(0, H), "z": slice(H, 2 * H), "n": slice(2 * H, 3 * H)}

    def strip_pack(cols, width):
        """list of 4 (H or 2, width*4?) -> (rows, 4*width) laid per strip"""
        rows = cols[0].shape[0]
        out = np.zeros((rows, 4 * width), f)
        for q in range(4):
            for i, Mfull in enumerate(cols):
                out[:, width * q + 128 * i:width * q + 128 * (i + 1)] = \
                    Mfull[:, 128 * q:128 * q + 128]
        return out

    # AR moving: per strip [r | z(neg) | hn | in]
    A = [W_hh.T[:, sl["r"]] + W_gi[:, sl["r"]],
         -(W_hh.T[:, sl["z"]] + W_gi[:, sl["z"]]),
         W_hh.T[:, sl["n"]],
         W_gi[:, sl["n"]]]
    w_ar = strip_pack(A, 512)                       # (512, 2048)
    w_ar = w_ar.reshape(4, 128, 2048).transpose(1, 0, 2)

    # TF moving: per strip [r | z(neg) | hn]
    Tm = [W_hh.T[:, sl["r"]], -W_hh.T[:, sl["z"]], W_hh.T[:, sl["n"]]]
    w_tf = strip_pack(Tm, 384)                      # (512, 1536)
    w_tf = w_tf.reshape(4, 128, 1536).transpose(1, 0, 2)

    zero_h = np.zeros((1, H), f)
    # K4 rows: row0 = delta coefs, row1 = bias coefs (per strip [r|z|hn|in])
    w4_ar = np.concatenate([
        strip_pack([w_last[None, sl["r"]], -w_last[None, sl["z"]],
                    zero_h, w_last[None, sl["n"]]], 512),
        strip_pack([(b_hh[sl["r"]] + c_ar[sl["r"]])[None],
                    -(b_hh[sl["z"]] + c_ar[sl["z"]])[None],
                    b_hh[None, sl["n"]],
                    c_ar[None, sl["n"]]], 512),
    ], axis=0)
    w4_tf = np.concatenate([
        strip_pack([w_last[None, sl["r"]], -w_last[None, sl["z"]],
                    zero_h, w_last[None, sl["n"]]], 512),
        strip_pack([(b_ih[sl["r"]] + b_hh[sl["r"]])[None],
                    -(b_ih[sl["z"]] + b_hh[sl["z"]])[None],
                    b_hh[None, sl["n"]],
                    b_ih[None, sl["n"]]], 512),
    ], axis=0)

    # x-side for TF: per strip [r | z(neg) | n] (first 256 = rz, last 128 = n)
    w_x = strip_pack([Wih_d.T[:, sl["r"]], -Wih_d.T[:, sl["z"]],
                      Wih_d.T[:, sl["n"]]], 384)    # (64, 1536)
    w_x = w_x.reshape(D, 4, 384)

    w_out_k = W_out.reshape(4, 128, D).transpose(1, 0, 2)
    b_out_c = b_out.reshape(D, 1)
    ident = np.eye(128, dtype=f)

    return dict(
        w_ar=np.ascontiguousarray(w_ar), w_tf=np.ascontiguousarray(w_tf),
        w4_ar=np.ascontiguousarray(w4_ar), w4_tf=np.ascontiguousarray(w4_tf),
        w_x=np.ascontiguousarray(w_x),
        w_out_k=np.ascontiguousarray(w_out_k),
        b_out_c=np.ascontiguousarray(b_out_c), ident=ident,
    )


def kernel(x, tp, samp_mask, W_ih, W_hh, b_ih, b_hh, W_out, b_out,
           _trace=False):
    from concourse.bass_utils import run_bass_kernel_spmd

    x = np.ascontiguousarray(np.asarray(x, np.float32))
    tp = np.ascontiguousarray(np.asarray(tp, np.float32))
    samp_mask = np.asarray(samp_mask)

    key = tuple(int(v) for v in samp_mask)
    if key not in _CACHE:
        _CACHE[key] = _build(samp_mask)
    nc = _CACHE[key]

    import ml_dtypes
    bf = ml_dtypes.bfloat16
    wdict = _preprocess(W_ih, W_hh, b_ih, b_hh, W_out, b_out)
    for k in ("w_ar", "w_tf", "w4_ar", "w4_tf", "w_x", "w_out_k"):
        wdict[k] = np.ascontiguousarray(wdict[k].astype(bf))

    # delta_t per (b, t); AR steps use x[:, t, 63] as the last channel
    offset = np.concatenate([np.zeros((B, 1), np.float32), tp[:, :-1]], axis=1)
    delta = tp - offset                       # (B, T)
    x63 = x[:, :, D - 1]                      # (B, T)
    m = samp_mask.astype(bool)[None, :]       # (1, T)
    aux = np.where(m, delta, x63).astype(np.float32)   # (B, T)

    in_maps = []
    for c in range(NC):
        mdict = dict(wdict)
        xl = x[c * BL:(c + 1) * BL]           # (BL, T, D)
        mdict["x_all"] = np.ascontiguousarray(
            xl.transpose(2, 1, 0).reshape(D, T * BL).astype(bf))
        dstat = np.ones((2, T * 32), np.float32)
        dstat[0].reshape(T, 32)[:, :BL] = aux[c * BL:(c + 1) * BL].T
        mdict["dstat"] = np.ascontiguousarray(dstat.astype(bf))
        in_maps.append(mdict)

    res = run_bass_kernel_spmd(nc, in_maps, list(range(NC)), trace=_trace)
    outs = []
    for c in range(NC):
        r = res.results[c]["out"].reshape(D, T, BL)   # [d, t, b]
        outs.append(r.transpose(2, 1, 0).reshape(BL * T, D))
    out = np.concatenate(outs, axis=0)
    if _trace:
        kernel.last_results = res
    return out.astype(np.float32)
